# revision 7
# baseline (speedup 1.0000x reference)
"""Trainium2 Bass kernel for nn_CAML_53240414601378.

Embedding lookup -> Conv1d(k=4, pad=2) -> tanh -> per-label attention
pooling -> logits. Data-parallel over batch across 8 NeuronCores
(4 batches per core); small params replicated.

Gather strategy: SWDGE descriptor generation on the Pool engine is the
serial bottleneck (~9ns/row on one queue). Transpose-mode gathers
cannot overlap (they share the xbar transpose stream), so instead:
- dma_gather(transpose=False) chunks run on all 4 SWDGE queues
  concurrently (each queue = its own pair of GpSimd cores, ~4x the
  generation rate). Each gathered row lands contiguously in one
  partition: XR[p, c, e] = emb[ids[c*128+p]][e].
- One hwdge dma_start(transpose=True) per chunk (Sync engine, xbar)
  converts XR into conv layout XT[e%128, c, e//128, s%128] - a blocked
  128x128 transpose per (c, E-half).
- conv1d(k=4) = shifted bf16 matmuls; moving operands are 128-aligned
  pieces of XT with shifted PSUM column offsets. Tap-2 pieces exactly
  tile the psum range and carry start=True.
- scores = U_w @ H and t = final_w @ H from ONE matmul per (F-chunk,
  seq-tile) with a combined stationary operand.
- softmax needs no max subtraction (|scores| < 0.2): Z comes free via
  the Exp activation accumulator, num = sum(exp(s) * t); logits =
  num/Z + final_b.
"""

import numpy as np
import ml_dtypes

import concourse.bass as bass
import concourse.tile as tile
from concourse.tile import add_dep_helper
from concourse import bacc, mybir
from concourse.bass_utils import run_bass_kernel_spmd

B, S = 32, 4096
VOCAB, E, F, L = 30522, 256, 256, 50
SO = S + 1  # conv output length (4097)
N_CORES = 8
BPC = B // N_CORES  # batches per core
BF16 = mybir.dt.bfloat16
FP32 = mybir.dt.float32
NT, TN = 8, 512  # full seq tiles covering t in [0, 4096)
NJ = NT + 1      # score tiles (8x512 + 1)
N_Q = 4          # SWDGE queues used for gather descriptor generation

_cache = {}


def _plan(b):
    if b == 0:
        return [1024, 1024, 2048]
    if b == BPC - 1:
        return [2048, 1536, 512]
    return [2048, 2048]


def _pieces(lo, hi, segs):
    """Split batch-col range [lo, hi) into matmul moving pieces.

    Each piece is (seg_idx, blk, r, w, off): chunk seg, 128-block index
    within the chunk, start within block, width (within one block or
    whole blocks when r == 0 and w % 128 == 0), psum offset (cols from
    `lo`).
    """
    out = []
    for si, (g0, g1) in enumerate(segs):
        a, bnd = max(lo, g0), min(hi, g1)
        while a < bnd:
            r = (a - g0) % 128
            blk = (a - g0) // 128
            if r:
                w = min(128 - r, bnd - a)
            else:
                nb = (bnd - a) // 128
                w = nb * 128 if nb else bnd - a
            out.append((si, blk, r, w, a - lo))
            a += w
    return out


def build_nc():
    nc = bacc.Bacc("TRN2", target_bir_lowering=False, debug=False,
                   num_devices=N_CORES, num_swdge_queues=N_Q)

    emb_ap = nc.dram_tensor("emb", (VOCAB, E), BF16, kind="ExternalInput").ap()
    idx_ap = nc.dram_tensor("idx", (128, BPC * S // 16), mybir.dt.int16,
                            kind="ExternalInput").ap()
    pb_ap = nc.dram_tensor("pbf", (128, 18, 128), BF16,
                           kind="ExternalInput").ap()
    pf_ap = nc.dram_tensor("pfp", (128, 4), FP32, kind="ExternalInput").ap()
    out_ap = nc.dram_tensor("out", (L, BPC), FP32, kind="ExternalOutput").ap()

    with tile.TileContext(nc) as tc:
        with (
            tc.tile_pool(name="const", bufs=1) as const,
            tc.tile_pool(name="xr", bufs=1) as xr,     # raw gather chunks
            tc.tile_pool(name="xt", bufs=1) as xtp,    # transposed chunks
            tc.tile_pool(name="hp", bufs=2) as hp,
            tc.tile_pool(name="ep", bufs=3) as ep,     # exp scratch tiles
            tc.tile_pool(name="pp", bufs=2) as pp,     # per-batch partials
            tc.tile_pool(name="small", bufs=8) as small,
            tc.tile_pool(name="psum", bufs=2, space="PSUM") as psum,
            tc.tile_pool(name="psum_st", bufs=4, space="PSUM") as psum_st,
        ):
            # ---- constants (loaded once, off the Pool engine) ----
            idx_sb = const.tile([128, BPC * S // 16], mybir.dt.int16)
            nc.sync.dma_start(idx_sb[:], idx_ap[:])
            pb_sb = const.tile([128, 18, 128], BF16)
            nc.sync.dma_start(pb_sb[:], pb_ap[:])
            pf_sb = const.tile([128, 4], FP32)
            nc.sync.dma_start(pf_sb[:], pf_ap[:])
            w_sb = pb_sb        # slots 0..15: conv weights
            cb_sb = pf_sb       # cols 0..1: conv bias
            out_sb = const.tile([L, BPC], FP32)

            IPB = S // 16  # idx columns per batch
            prev_q = {}    # queue -> last gather inst (ring order per queue)
            qi = 0

            for b in range(BPC):
                # ---- gather (no transpose) + xbar transpose per chunk ----
                plan = _plan(b)
                segs = []       # (g0, g1)
                xts = []        # transposed tiles [128, nb, 2, 128]
                g0 = 0
                for ci, cs in enumerate(plan):
                    nb = cs // 128
                    q = qi % N_Q
                    qi += 1
                    xrt = xr.tile([128, nb, E], BF16, tag=f"r{b}c{ci}")
                    gi = nc.gpsimd.dma_gather(
                        out_ap=xrt[:], in_ap=emb_ap[:],
                        idxs_ap=idx_sb[:, b * IPB + g0 // 16:
                                       b * IPB + (g0 + cs) // 16],
                        num_idxs=cs, num_idxs_reg=cs, elem_size=E,
                        transpose=False, single_packet=False, queue_num=q)
                    if q in prev_q:
                        add_dep_helper(prev_q[q].ins, gi.ins, False,
                                       "per-queue gather order")
                    prev_q[q] = gi
                    xtt = xtp.tile([128, nb, 2, 128], BF16, tag=f"b{b}c{ci}")
                    nc.sync.dma_start(xtt[:], xrt[:], transpose=True)
                    segs.append((g0, g0 + cs))
                    xts.append(xtt)
                    g0 += cs

                def moving(si, blk, r, w):
                    """AP for piece: XT[e_lo, blocks, eh, s_lo] slice."""
                    xtt = xts[si]
                    if r == 0 and w % 128 == 0 and w > 0:
                        return (xtt[:, blk:blk + w // 128, 0, :],
                                xtt[:, blk:blk + w // 128, 1, :])
                    return (xtt[:, blk, 0, r:r + w],
                            xtt[:, blk, 1, r:r + w])

                H = hp.tile([128, 2, SO], BF16, tag="H")
                zp = pp.tile([L, NJ], FP32, tag="zp")    # partial Z
                np_ = pp.tile([L, NJ], FP32, tag="np")   # partial num

                def score_tile(j, n):
                    t0 = j * TN
                    pst = psum_st.tile([114, TN], FP32, tag="st")
                    for fc in range(2):
                        nc.tensor.matmul(
                            pst[:, 0:n], pb_sb[:, 16 + fc, 0:114],
                            H[:, fc, t0:t0 + n],
                            start=(fc == 0), stop=(fc == 1),
                        )
                    e_sb = ep.tile([L, TN], FP32, tag="e")
                    nc.scalar.activation(
                        e_sb[:, 0:n], pst[0:L, 0:n],
                        mybir.ActivationFunctionType.Exp,
                        accum_out=zp[:, j:j + 1],
                    )
                    nc.vector.tensor_mul(e_sb[:, 0:n], e_sb[:, 0:n],
                                         pst[64:64 + L, 0:n])
                    nc.vector.reduce_sum(np_[:, j:j + 1], e_sb[:, 0:n],
                                         axis=mybir.AxisListType.X)

                # ---- conv1d(k=4) + bias + tanh + scores, per seq tile ----
                for j in range(NT):
                    t0 = j * TN
                    # per tap: clipped input range and psum base offset
                    mms = []  # (piece_tuple, k)
                    for k in (2, 0, 1, 3):  # tap 2 first: start=True cover
                        lo = max(0, t0 + k - 2)
                        hi = min(S, t0 + k - 2 + TN)
                        base = lo - (t0 + k - 2)
                        for (si, blk, r, w, off) in _pieces(lo, hi, segs):
                            mms.append((si, blk, r, w, base + off, k))
                    n2 = sum(1 for m in mms if m[5] == 2)
                    assert n2 == 1, (b, j, n2)  # single full-width start
                    for fc in range(2):
                        ph = psum.tile([128, TN], FP32, tag=f"h{fc}")
                        nmm = len(mms) * 2
                        i = 0
                        for (si, blk, r, w, off, k) in mms:
                            mv = moving(si, blk, r, w)
                            for ec in range(2):
                                nc.tensor.matmul(
                                    ph[:, off:off + w],
                                    w_sb[:, k * 4 + ec * 2 + fc, :],
                                    mv[ec],
                                    start=(i < n2 * 2 and ec == 0),
                                    stop=(i == nmm - 1),
                                )
                                i += 1
                        nc.scalar.activation(
                            H[:, fc, t0:t0 + TN], ph[:],
                            mybir.ActivationFunctionType.Tanh,
                            bias=cb_sb[:, fc:fc + 1],
                        )
                    score_tile(j, TN)

                # last output column t = 4096 (x cols 4094..4095, taps 0..1)
                for fc in range(2):
                    ph9 = psum.tile([128, 1], FP32, tag=f"h{fc}")
                    i = 0
                    for k in range(2):
                        (si, blk, r, w, off) = _pieces(4094 + k, 4095 + k,
                                                       segs)[0]
                        mv = moving(si, blk, r, w)
                        for ec in range(2):
                            nc.tensor.matmul(
                                ph9[:, 0:1],
                                w_sb[:, k * 4 + ec * 2 + fc, :],
                                mv[ec],
                                start=(i == 0), stop=(i == 3),
                            )
                            i += 1
                    nc.scalar.activation(
                        H[:, fc, S:SO], ph9[:],
                        mybir.ActivationFunctionType.Tanh,
                        bias=cb_sb[:, fc:fc + 1],
                    )
                score_tile(NT, 1)

                # ---- combine partials -> logits ----
                zsum = small.tile([L, 1], FP32, tag="zsum")
                nc.vector.reduce_sum(zsum[:], zp[:], axis=mybir.AxisListType.X)
                nsum = small.tile([L, 1], FP32, tag="nsum")
                nc.vector.reduce_sum(nsum[:], np_[:], axis=mybir.AxisListType.X)
                zr = small.tile([L, 1], FP32, tag="zr")
                nc.vector.reciprocal(zr[:], zsum[:])
                sm = small.tile([L, 1], FP32, tag="sm")
                nc.vector.tensor_mul(sm[:], nsum[:], zr[:])
                nc.vector.tensor_add(out_sb[:, b:b + 1], sm[:],
                                     pf_sb[0:L, 2:3])

            nc.sync.dma_start(out_ap[:], out_sb[:])

    nc.compile()
    return nc


def _prep_shared(emb_table, conv_w, conv_b, U_w, final_w, final_b):
    emb_bf = np.ascontiguousarray(emb_table.astype(ml_dtypes.bfloat16))

    # wconv[e_lo, k*4 + ec*2 + fc, f_lo] = conv_w[fc*128+f, ec*128+e, k]
    W = np.empty((128, 16, 128), np.float32)
    for k in range(4):
        for ec in range(2):
            for fc in range(2):
                W[:, k * 4 + ec * 2 + fc, :] = conv_w[
                    fc * 128:(fc + 1) * 128, ec * 128:(ec + 1) * 128, k].T
    # packed bf16 params: slots 0..15 conv weights, 16..17 uwfw
    # uwfw[f_lo, fc, j]: j<50 -> U_w[j, fc*128+f_lo];
    # j in [64,114) -> final_w[j-64, fc*128+f_lo]; rest zero
    PB = np.zeros((128, 18, 128), np.float32)
    PB[:, 0:16, :] = W
    PB[:, 16:18, 0:L] = U_w.T.reshape(2, 128, L).transpose(1, 0, 2)
    PB[:, 16:18, 64:64 + L] = final_w.T.reshape(2, 128, L).transpose(1, 0, 2)
    PB = np.ascontiguousarray(PB.astype(ml_dtypes.bfloat16))

    # packed fp32 params: cols 0..1 conv bias, col 2 rows 0..49 final bias
    PF = np.zeros((128, 4), np.float32)
    PF[:, 0:2] = conv_b.reshape(2, 128).T
    PF[0:L, 2] = final_b
    PF = np.ascontiguousarray(PF)
    return emb_bf, PB, PF


def kernel(input_ids, emb_table, conv_w, conv_b, U_w, final_w, final_b):
    import os
    ids = np.asarray(input_ids)
    emb_table = np.asarray(emb_table, dtype=np.float32)
    conv_w = np.asarray(conv_w, dtype=np.float32)
    conv_b = np.asarray(conv_b, dtype=np.float32)
    U_w = np.asarray(U_w, dtype=np.float32)
    final_w = np.asarray(final_w, dtype=np.float32)
    final_b = np.asarray(final_b, dtype=np.float32)

    if "nc" not in _cache:
        _cache["nc"] = build_nc()
    nc = _cache["nc"]

    emb_bf, PB, PF = _prep_shared(
        emb_table, conv_w, conv_b, U_w, final_w, final_b)

    ids16 = ids.astype(np.int16)  # vocab 30522 < 2**15
    in_maps = []
    for c in range(N_CORES):
        cid = ids16[c * BPC:(c + 1) * BPC]  # (BPC, S)
        # position i -> [i % 16, i // 16], batches along axis 1; the
        # 16-row block is replicated to all 8 gpsimd cores (128 rows)
        blk = np.concatenate(
            [cid[b].reshape(S // 16, 16).T for b in range(BPC)], axis=1)
        idx = np.tile(blk, (8, 1))
        in_maps.append({
            "emb": emb_bf, "idx": np.ascontiguousarray(idx),
            "pbf": PB, "pfp": PF,
        })

    trace = bool(int(os.environ.get("KERNEL_TRACE", "0")))
    res = run_bass_kernel_spmd(nc, in_maps, core_ids=list(range(N_CORES)),
                               trace=trace)
    _cache["last_result"] = res

    out = np.concatenate(
        [res.results[c]["out"].T for c in range(N_CORES)], axis=0)
    return np.ascontiguousarray(out.astype(np.float32))


# revision 12
# speedup vs baseline: 1.5570x; 1.5570x over previous
"""Trainium2 Bass kernel for nn_CAML_53240414601378.

Embedding lookup -> Conv1d(k=4, pad=2) -> tanh -> per-label attention
pooling -> logits. Data-parallel over batch across 8 NeuronCores
(4 batches per core); small params replicated.

Gather strategy: SWDGE descriptor generation on the Pool engine is the
serial bottleneck (~9ns/row on one queue). Transpose-mode gathers
cannot overlap (they share the xbar transpose stream), so instead:
- dma_gather(transpose=False) chunks run on all 4 SWDGE queues
  concurrently (each queue = its own pair of GpSimd cores, ~4x the
  generation rate). Each gathered row lands contiguously in one
  partition: XR[p, c, e] = emb[ids[c*128+p]][e].
- One hwdge dma_start(transpose=True) per chunk (Sync engine, xbar)
  converts XR into conv layout XT[e%128, c, e//128, s%128] - a blocked
  128x128 transpose per (c, E-half).
- conv1d(k=4) = shifted bf16 matmuls; moving operands are 128-aligned
  pieces of XT with shifted PSUM column offsets. Tap-2 pieces exactly
  tile the psum range and carry start=True.
- scores = U_w @ H and t = final_w @ H from ONE matmul per (F-chunk,
  seq-tile) with a combined stationary operand.
- softmax needs no max subtraction (|scores| < 0.2): Z comes free via
  the Exp activation accumulator, num = sum(exp(s) * t); logits =
  num/Z + final_b.
"""

import numpy as np
import ml_dtypes

import concourse.bass as bass
import concourse.tile as tile
from concourse.tile import add_dep_helper
from concourse import bacc, mybir
from concourse.bass_utils import run_bass_kernel_spmd

B, S = 32, 4096
VOCAB, E, F, L = 30522, 256, 256, 50
SO = S + 1  # conv output length (4097)
N_CORES = 8
BPC = B // N_CORES  # batches per core
BF16 = mybir.dt.bfloat16
FP32 = mybir.dt.float32
NT, TN = 8, 512  # full seq tiles covering t in [0, 4096)
NJ = NT + 1      # score tiles (8x512 + 1)
N_Q = 4          # SWDGE queues used for gather descriptor generation

_cache = {}


def _plan(b):
    if b == 0:
        return [1024, 1024, 2048]
    if b == BPC - 1:
        return [2048, 1536, 512]
    return [2048, 2048]


def _pieces(lo, hi, segs):
    """Split batch-col range [lo, hi) into matmul moving pieces.

    Each piece is (seg_idx, blk, r, w, off): chunk seg, 128-block index
    within the chunk, start within block, width (within one block or
    whole blocks when r == 0 and w % 128 == 0), psum offset (cols from
    `lo`).
    """
    out = []
    for si, (g0, g1) in enumerate(segs):
        a, bnd = max(lo, g0), min(hi, g1)
        while a < bnd:
            r = (a - g0) % 128
            blk = (a - g0) // 128
            if r:
                w = min(128 - r, bnd - a)
            else:
                nb = (bnd - a) // 128
                w = nb * 128 if nb else bnd - a
            out.append((si, blk, r, w, a - lo))
            a += w
    return out


def build_nc():
    nc = bacc.Bacc("TRN2", target_bir_lowering=False, debug=False,
                   num_devices=N_CORES, num_swdge_queues=N_Q,
                   dynamic_dma_scratch_size=49152)

    emb_ap = nc.dram_tensor("emb", (VOCAB, E), BF16, kind="ExternalInput").ap()
    idx_ap = nc.dram_tensor("idx", (128, BPC * S // 16), mybir.dt.int16,
                            kind="ExternalInput").ap()
    pb_ap = nc.dram_tensor("pbf", (128, 18, 128), BF16,
                           kind="ExternalInput").ap()
    pf_ap = nc.dram_tensor("pfp", (128, 4), FP32, kind="ExternalInput").ap()
    out_ap = nc.dram_tensor("out", (L, BPC), FP32, kind="ExternalOutput").ap()

    with tile.TileContext(nc) as tc:
        with (
            tc.tile_pool(name="const", bufs=1) as const,
            tc.tile_pool(name="hp", bufs=2) as hp,
            tc.tile_pool(name="ep", bufs=3) as ep,     # exp scratch tiles
            tc.tile_pool(name="pp", bufs=2) as pp,     # per-batch partials
            tc.tile_pool(name="small", bufs=8) as small,
            tc.tile_pool(name="psum", bufs=2, space="PSUM") as psum,
            tc.tile_pool(name="psum_st", bufs=4, space="PSUM") as psum_st,
        ):
            # ---- constants (loaded once, off the Pool engine) ----
            idx_sb = const.tile([128, BPC * S // 16], mybir.dt.int16)
            nc.sync.dma_start(idx_sb[:], idx_ap[:])
            pb_sb = const.tile([128, 18, 128], BF16)
            nc.sync.dma_start(pb_sb[:], pb_ap[:])
            pf_sb = const.tile([128, 4], FP32)
            nc.sync.dma_start(pf_sb[:], pf_ap[:])
            w_sb = pb_sb        # slots 0..15: conv weights
            cb_sb = pf_sb       # cols 0..1: conv bias
            out_sb = const.tile([L, BPC], FP32)
            # immortal staging buffers, manually slot-rotated (keeps the
            # autobufs packer from aliasing transient tiles, which would
            # bake the simulator's serial schedule into sem deps)
            xr_all = const.tile([128, 64, E], BF16)      # raw gather rows
            xt_all = const.tile([128, 64, 2, 128], BF16)  # transposed

            IPB = S // 16  # idx columns per batch
            prev_q = {}    # queue -> last gather inst (ring order per queue)
            qi = 0
            xr_off = [0]   # rotating block slot in xr_all
            # distinct num_idxs register per gather: a shared to_reg()
            # register is a serial resource that entangles concurrent
            # gathers across queues
            nregs = {}
            for b_ in range(BPC):
                for ci_, cs_ in enumerate(_plan(b_)):
                    if cs_ not in nregs:
                        r_ = nc.gpsimd.alloc_register(f"nidx{cs_}")
                        nc.gpsimd.reg_mov(r_, cs_)
                        nregs[cs_] = r_
            xr_readers = []    # (lo, hi, xpose inst) for live xr_all slots
            last_conv_mm = {}  # batch -> last conv matmul (last xt reader)

            for b in range(BPC):
                # ---- gather (no transpose) + xbar transpose per chunk ----
                plan = _plan(b)
                segs = []       # (g0, g1)
                xt_base = 32 * (b % 2)
                g0 = 0
                for ci, cs in enumerate(plan):
                    nb = cs // 128
                    q = qi % N_Q
                    qi += 1
                    if xr_off[0] + nb > 64:
                        xr_off[0] = 0
                    ro = xr_off[0]
                    xr_off[0] += nb
                    xrt = xr_all[:, ro:ro + nb, :]
                    gi = nc.gpsimd.dma_gather(
                        out_ap=xrt, in_ap=emb_ap[:],
                        idxs_ap=idx_sb[:, b * IPB + g0 // 16:
                                       b * IPB + (g0 + cs) // 16],
                        num_idxs=cs, num_idxs_reg=nregs[cs], elem_size=E,
                        transpose=False, single_packet=False, queue_num=q)
                    if q in prev_q:
                        add_dep_helper(gi.ins, prev_q[q].ins, False,
                                       "per-queue gather order")
                    prev_q[q] = gi
                    # explicit WAR: overwriting an xr_all slot must wait
                    # for the xpose that last read it (the scheduler does
                    # not model slice-level WARs on shared tiles)
                    keep = []
                    for (lo_, hi_, xp_) in xr_readers:
                        if lo_ < ro + nb and ro < hi_:
                            add_dep_helper(gi.ins, xp_.ins, True,
                                           "xr slot rotation")
                        else:
                            keep.append((lo_, hi_, xp_))
                    cb0 = xt_base + g0 // 128
                    xpi = nc.sync.dma_start(xt_all[:, cb0:cb0 + nb, :, :],
                                            xrt, transpose=True)
                    if b >= 2 and (b - 2) in last_conv_mm:
                        add_dep_helper(xpi.ins, last_conv_mm[b - 2].ins,
                                       True, "xt slot rotation")
                    xr_readers = keep + [(ro, ro + nb, xpi)]
                    segs.append((g0, g0 + cs))
                    g0 += cs

                def moving(si, blk, r, w):
                    """AP for piece: XT[e_lo, blocks, eh, s_lo] slice."""
                    g0s, _ = segs[si]
                    b0 = xt_base + g0s // 128 + blk
                    if r == 0 and w % 128 == 0 and w > 0:
                        return (xt_all[:, b0:b0 + w // 128, 0, :],
                                xt_all[:, b0:b0 + w // 128, 1, :])
                    return (xt_all[:, b0, 0, r:r + w],
                            xt_all[:, b0, 1, r:r + w])

                H = hp.tile([128, 2, SO], BF16, tag="H")
                zp = pp.tile([L, NJ], FP32, tag="zp")    # partial Z
                np_ = pp.tile([L, NJ], FP32, tag="np")   # partial num

                def score_tile(j, n):
                    t0 = j * TN
                    pst = psum_st.tile([114, TN], FP32, tag="st")
                    for fc in range(2):
                        nc.tensor.matmul(
                            pst[:, 0:n], pb_sb[:, 16 + fc, 0:114],
                            H[:, fc, t0:t0 + n],
                            start=(fc == 0), stop=(fc == 1),
                        )
                    e_sb = ep.tile([L, TN], FP32, tag="e")
                    nc.scalar.activation(
                        e_sb[:, 0:n], pst[0:L, 0:n],
                        mybir.ActivationFunctionType.Exp,
                        accum_out=zp[:, j:j + 1],
                    )
                    nc.vector.tensor_mul(e_sb[:, 0:n], e_sb[:, 0:n],
                                         pst[64:64 + L, 0:n])
                    nc.vector.reduce_sum(np_[:, j:j + 1], e_sb[:, 0:n],
                                         axis=mybir.AxisListType.X)

                # ---- conv1d(k=4) + bias + tanh + scores, per seq tile ----
                for j in range(NT):
                    t0 = j * TN
                    # per tap: clipped input range and psum base offset
                    mms = []  # (piece_tuple, k)
                    for k in (2, 0, 1, 3):  # tap 2 first: start=True cover
                        lo = max(0, t0 + k - 2)
                        hi = min(S, t0 + k - 2 + TN)
                        base = lo - (t0 + k - 2)
                        for (si, blk, r, w, off) in _pieces(lo, hi, segs):
                            mms.append((si, blk, r, w, base + off, k))
                    n2 = sum(1 for m in mms if m[5] == 2)
                    assert n2 == 1, (b, j, n2)  # single full-width start
                    for fc in range(2):
                        ph = psum.tile([128, TN], FP32, tag=f"h{fc}")
                        nmm = len(mms) * 2
                        i = 0
                        for (si, blk, r, w, off, k) in mms:
                            mv = moving(si, blk, r, w)
                            for ec in range(2):
                                nc.tensor.matmul(
                                    ph[:, off:off + w],
                                    w_sb[:, k * 4 + ec * 2 + fc, :],
                                    mv[ec],
                                    start=(i < n2 * 2 and ec == 0),
                                    stop=(i == nmm - 1),
                                )
                                i += 1
                        nc.scalar.activation(
                            H[:, fc, t0:t0 + TN], ph[:],
                            mybir.ActivationFunctionType.Tanh,
                            bias=cb_sb[:, fc:fc + 1],
                        )
                    score_tile(j, TN)

                # last output column t = 4096 (x cols 4094..4095, taps 0..1)
                for fc in range(2):
                    ph9 = psum.tile([128, 1], FP32, tag=f"h{fc}")
                    i = 0
                    for k in range(2):
                        (si, blk, r, w, off) = _pieces(4094 + k, 4095 + k,
                                                       segs)[0]
                        mv = moving(si, blk, r, w)
                        for ec in range(2):
                            last_conv_mm[b] = nc.tensor.matmul(
                                ph9[:, 0:1],
                                w_sb[:, k * 4 + ec * 2 + fc, :],
                                mv[ec],
                                start=(i == 0), stop=(i == 3),
                            )
                            i += 1
                    nc.scalar.activation(
                        H[:, fc, S:SO], ph9[:],
                        mybir.ActivationFunctionType.Tanh,
                        bias=cb_sb[:, fc:fc + 1],
                    )
                score_tile(NT, 1)

                # ---- combine partials -> logits ----
                zsum = small.tile([L, 1], FP32, tag="zsum")
                nc.vector.reduce_sum(zsum[:], zp[:], axis=mybir.AxisListType.X)
                nsum = small.tile([L, 1], FP32, tag="nsum")
                nc.vector.reduce_sum(nsum[:], np_[:], axis=mybir.AxisListType.X)
                zr = small.tile([L, 1], FP32, tag="zr")
                nc.vector.reciprocal(zr[:], zsum[:])
                sm = small.tile([L, 1], FP32, tag="sm")
                nc.vector.tensor_mul(sm[:], nsum[:], zr[:])
                nc.vector.tensor_add(out_sb[:, b:b + 1], sm[:],
                                     pf_sb[0:L, 2:3])

            nc.sync.dma_start(out_ap[:], out_sb[:])

    nc.compile()
    return nc


def _prep_shared(emb_table, conv_w, conv_b, U_w, final_w, final_b):
    emb_bf = np.ascontiguousarray(emb_table.astype(ml_dtypes.bfloat16))

    # wconv[e_lo, k*4 + ec*2 + fc, f_lo] = conv_w[fc*128+f, ec*128+e, k]
    W = np.empty((128, 16, 128), np.float32)
    for k in range(4):
        for ec in range(2):
            for fc in range(2):
                W[:, k * 4 + ec * 2 + fc, :] = conv_w[
                    fc * 128:(fc + 1) * 128, ec * 128:(ec + 1) * 128, k].T
    # packed bf16 params: slots 0..15 conv weights, 16..17 uwfw
    # uwfw[f_lo, fc, j]: j<50 -> U_w[j, fc*128+f_lo];
    # j in [64,114) -> final_w[j-64, fc*128+f_lo]; rest zero
    PB = np.zeros((128, 18, 128), np.float32)
    PB[:, 0:16, :] = W
    PB[:, 16:18, 0:L] = U_w.T.reshape(2, 128, L).transpose(1, 0, 2)
    PB[:, 16:18, 64:64 + L] = final_w.T.reshape(2, 128, L).transpose(1, 0, 2)
    PB = np.ascontiguousarray(PB.astype(ml_dtypes.bfloat16))

    # packed fp32 params: cols 0..1 conv bias, col 2 rows 0..49 final bias
    PF = np.zeros((128, 4), np.float32)
    PF[:, 0:2] = conv_b.reshape(2, 128).T
    PF[0:L, 2] = final_b
    PF = np.ascontiguousarray(PF)
    return emb_bf, PB, PF


def kernel(input_ids, emb_table, conv_w, conv_b, U_w, final_w, final_b):
    import os
    ids = np.asarray(input_ids)
    emb_table = np.asarray(emb_table, dtype=np.float32)
    conv_w = np.asarray(conv_w, dtype=np.float32)
    conv_b = np.asarray(conv_b, dtype=np.float32)
    U_w = np.asarray(U_w, dtype=np.float32)
    final_w = np.asarray(final_w, dtype=np.float32)
    final_b = np.asarray(final_b, dtype=np.float32)

    if "nc" not in _cache:
        _cache["nc"] = build_nc()
    nc = _cache["nc"]

    emb_bf, PB, PF = _prep_shared(
        emb_table, conv_w, conv_b, U_w, final_w, final_b)

    ids16 = ids.astype(np.int16)  # vocab 30522 < 2**15
    in_maps = []
    for c in range(N_CORES):
        cid = ids16[c * BPC:(c + 1) * BPC]  # (BPC, S)
        # position i -> [i % 16, i // 16], batches along axis 1; the
        # 16-row block is replicated to all 8 gpsimd cores (128 rows)
        blk = np.concatenate(
            [cid[b].reshape(S // 16, 16).T for b in range(BPC)], axis=1)
        idx = np.tile(blk, (8, 1))
        in_maps.append({
            "emb": emb_bf, "idx": np.ascontiguousarray(idx),
            "pbf": PB, "pfp": PF,
        })

    trace = bool(int(os.environ.get("KERNEL_TRACE", "0")))
    res = run_bass_kernel_spmd(nc, in_maps, core_ids=list(range(N_CORES)),
                               trace=trace)
    _cache["last_result"] = res

    out = np.concatenate(
        [res.results[c]["out"].T for c in range(N_CORES)], axis=0)
    return np.ascontiguousarray(out.astype(np.float32))
